# revision 18
# baseline (speedup 1.0000x reference)
"""EntropyProfileLoss Trainium2 kernel — TensorE windowed-sum design.

Math: for a window t of length k, sum(softmax(t)*log_softmax(t))
      = S2/S1 - ln(S1),  S1 = sum(exp(t)), S2 = sum(t*exp(t)).

Layout: pure data parallel over batch B=64 -> 8 cores x 8 batches
(16 rows of L=2048 per core).  Each row is split into 16 blocks of
128; the host transposes to a [128, 257] SBUF image whose partition
axis is the position-within-block j and whose free axis is
c = 16*row + block (col 256 is a -100 pad so exp() gives 0).  In this
layout a window sum starting at (j, c) is a sum down the partition
axis spilling into column c+1, which TensorE computes as two banded
matmuls per window size:

  S_k[q, c] = sum_{p=q}^{q+k-1} E[p, c] + sum_{p=0}^{q+k-129} E[p, c+1]
            = (A_k^T E)[q, c]          + (B_k^T E_next)[q, c]

with A_k[p,q] = 1_{q <= p < q+k}, B_k[p,q] = 1_{p < q+k-128} packed as
12 bf16 [128,128] weights (host inputs).  E = exp(x) and XE = x*E are
bf16; the matmuls accumulate S1 (from E) and S2 (from XE) in fp32
PSUM — one 1024-col region [S1x|S2x|S1t|S2t] per k, cycled through
the 8 PSUM banks (4 k's in flight).

Downstream per k: U = ln(S1) (ACT, PSUM-src, fp32), R = exp(-U) bf16
(ACT, batched over k pairs), D = S2*R (DVE, PSUM-src), dU = Ux-Ut
(GpSimd), dD/dx (DVE bf16 2x), invalid-window memsets (GpSimd: block
15 windows with q >= 129-k run past the row end), and a per-k-pair
|dx| sum via tensor_reduce(apply_absolute_value) into ACC[128, 6].
The host reduces ACC over cores/partitions and applies the per-k
1/(B*C*(L-k+1)) mean scaling.

DMA: x on the sync queue, t + the first 4 weights on the scalar
queue, remaining weights on sync after x — two HWDGE queues in
parallel.  ACT runs Exp and Ln from the single
natural_log_exp_and_others table set (see _patch_act_tables).
"""

import sys

import numpy as np

if "/opt/trn_rl_repo" not in sys.path:
    sys.path.insert(0, "/opt/trn_rl_repo")

import ml_dtypes

import concourse.bacc as bacc
import concourse.bass as bass
import concourse.tile as tile
from concourse import mybir

KERNELS = (4, 8, 16, 32, 64, 128)
NK = len(KERNELS)
B, C, L = 64, 2, 2048
N_CORES = 8
ROWS = (B // N_CORES) * C          # 16 rows per core
NB = L // 128                      # 16 blocks per row
COLS = ROWS * NB                   # 256 (free dim: c = 16*row + block)
NCOL = COLS + 1                    # +1 pad col (= -100 -> exp = 0)
PAD = -100.0

F32 = mybir.dt.float32
BF16 = mybir.dt.bfloat16
AF = mybir.ActivationFunctionType
OP = mybir.AluOpType

_CACHE: dict = {}


def _patch_act_tables():
    """Keep Exp/Ln resolvable only via natural_log_exp_and_others so the
    table-load pass emits one ACT table set (one ~2.7us load)."""
    if _CACHE.get("act_patched"):
        return
    orig = bacc.get_activation_tables
    funcs = {AF.Exp, AF.Ln}

    def patched(arch):
        tables = dict(orig(arch))
        return {
            name: (fs if name == "natural_log_exp_and_others" else fs - funcs)
            for name, fs in tables.items()
        }

    bacc.get_activation_tables = patched
    _CACHE["act_patched"] = True


def make_weights() -> np.ndarray:
    """[128, 2*NK, 128] bf16: A_k, B_k interleaved in k order."""
    p = np.arange(128)[:, None]
    q = np.arange(128)[None, :]
    w = np.zeros((128, 2 * NK, 128), dtype=np.float32)
    for ki, k in enumerate(KERNELS):
        w[:, 2 * ki, :] = ((q <= p) & (p < q + k)).astype(np.float32)
        w[:, 2 * ki + 1, :] = (p < q + k - 128).astype(np.float32)
    return w.astype(ml_dtypes.bfloat16)


def build(debug: bool = False, xe_on_gpsimd: bool = True,
          du_on_gpsimd: bool = True):
    _patch_act_tables()
    nc = bacc.Bacc("TRN2", target_bir_lowering=False)

    x_d = nc.dram_tensor("x", [128, NCOL], F32, kind="ExternalInput")
    t_d = nc.dram_tensor("t", [128, NCOL], F32, kind="ExternalInput")
    w_d = nc.dram_tensor("w", [128, 2 * NK * 128], BF16, kind="ExternalInput")
    acc_d = nc.dram_tensor("acc", [128, 2 * NK], F32, kind="ExternalOutput")
    if debug:
        ex_d = nc.dram_tensor("dbg_ex", [128, 4 * NCOL], BF16,
                              kind="ExternalOutput")
        u_d = nc.dram_tensor("dbg_u", [128, NK * 2 * COLS], F32,
                             kind="ExternalOutput")
        dx_d = nc.dram_tensor("dbg_dx", [128, NK * COLS], BF16,
                              kind="ExternalOutput")
        ps_d = nc.dram_tensor("dbg_ps", [128, NK * 1024], F32,
                              kind="ExternalOutput")
        d_d = nc.dram_tensor("dbg_d", [128, NK * 2 * COLS], BF16,
                             kind="ExternalOutput")

    with tile.TileContext(nc) as tc:
        with (
            tc.tile_pool(name="big", bufs=1) as big,
            tc.tile_pool(name="ps", bufs=1, space="PSUM") as psp,
        ):
            X = big.tile([128, 2, NCOL], F32)          # [x | t]
            EX = big.tile([128, 2, 2, NCOL], BF16)     # [tensor][E | XE]
            W = big.tile([128, 2 * NK, 128], BF16)
            U = big.tile([128, NK, 2, COLS], F32)
            R = big.tile([128, NK, 2, COLS], BF16)
            D = big.tile([128, NK, 2, COLS], BF16)
            dU = big.tile([128, NK, COLS], BF16)
            dD = big.tile([128, NK, COLS], BF16)
            dx = big.tile([128, NK, COLS], BF16)
            ACC = big.tile([128, 2, NK], F32)          # [main | block-15 tail]
            PS = psp.tile([128, 4096], F32)            # all 8 banks

            nc.vector.memset(ACC[:, :, :], 0.0)
            PSC = big.tile([128, NK, 1024], F32, name="PSC") if debug else None

            # ---- input DMA on two parallel HWDGE queues ----
            nc.sync.dma_start(out=X[:, 0, :], in_=x_d[:, :])
            nc.scalar.dma_start(out=X[:, 1, :], in_=t_d[:, :])
            wv = W[:, :, :].rearrange("p a b -> p (a b)")
            # first 4 weights (k=4,8) behind t on scalar; rest behind x on sync
            nc.scalar.dma_start(out=wv[:, 0 : 4 * 128], in_=w_d[:, 0 : 4 * 128])
            nc.sync.dma_start(out=wv[:, 4 * 128 :], in_=w_d[:, 4 * 128 :])

            # ---- E = exp(x) bf16, XE = x * E bf16 (pad col -> 0) ----
            for a in range(2):
                nc.scalar.activation(
                    out=EX[:, a, 0, :], in_=X[:, a, :], func=AF.Exp
                )
                xe_eng = nc.gpsimd if xe_on_gpsimd else nc.vector
                xe_eng.tensor_tensor(
                    out=EX[:, a, 1, :], in0=X[:, a, :], in1=EX[:, a, 0, :],
                    op=OP.mult,
                )

            # ---- per window size: banded matmuls + entropy pipeline ----
            for ki, k in enumerate(KERNELS):
                base = (ki % 4) * 1024
                reg = PS[:, base : base + 1024].rearrange(
                    "p (a s c) -> p a s c", a=2, s=2
                )  # [tensor][S1|S2][256]
                # one matmul per (weight, tensor) covering [E | XE] — FD 512
                # fills the bank exactly, so each bank sees one start/stop
                for wi, off, start in ((2 * ki, 0, True), (2 * ki + 1, 1, False)):
                    for a in range(2):
                        nc.tensor.matmul(
                            reg[:, a, :, :],
                            W[:, wi, :],
                            EX[:, a, :, off : off + COLS],
                            start=start,
                            stop=not start,
                        )
                if debug:
                    nc.vector.tensor_copy(
                        out=PSC[:, ki], in_=PS[:, base : base + 1024]
                    )
                nc.scalar.activation(
                    out=U[:, ki], in_=reg[:, :, 0, :], func=AF.Ln
                )
                du_eng = nc.gpsimd if du_on_gpsimd else nc.vector
                du_eng.tensor_tensor(
                    out=dU[:, ki], in0=U[:, ki, 0], in1=U[:, ki, 1],
                    op=OP.subtract,
                )
                if ki % 2 == 1:
                    # R = exp(-U) for the (ki-1, ki) pair
                    nc.scalar.activation(
                        out=R[:, ki - 1 : ki + 1], in_=U[:, ki - 1 : ki + 1],
                        func=AF.Exp, scale=-1.0,
                    )
                    for kj in (ki - 1, ki):
                        basej = (kj % 4) * 1024
                        regj = PS[:, basej : basej + 1024].rearrange(
                            "p (a s c) -> p a s c", a=2, s=2
                        )
                        nc.vector.tensor_tensor(
                            out=D[:, kj], in0=regj[:, :, 1, :], in1=R[:, kj],
                            op=OP.mult,
                        )
                    nc.vector.tensor_tensor(
                        out=dD[:, ki - 1 : ki + 1],
                        in0=D[:, ki - 1 : ki + 1, 0],
                        in1=D[:, ki - 1 : ki + 1, 1],
                        op=OP.subtract,
                    )
                    nc.vector.tensor_tensor(
                        out=dx[:, ki - 1 : ki + 1],
                        in0=dD[:, ki - 1 : ki + 1],
                        in1=dU[:, ki - 1 : ki + 1],
                        op=OP.subtract,
                    )
                    # |dx| sums, skipping invalid windows (block 15 windows
                    # with q >= 129-k run past the row end): main reduce over
                    # blocks 0-14, tail reduce over block 15's valid rows
                    dxv = dx[:, ki - 1 : ki + 1].rearrange(
                        "p kk (r b) -> p kk r b", b=NB
                    )
                    nc.vector.tensor_reduce(
                        out=ACC[:, 0, ki - 1 : ki + 1],
                        in_=dxv[:, :, :, 0 : NB - 1],
                        axis=mybir.AxisListType.XY,
                        op=OP.add,
                        apply_absolute_value=True,
                    )
                    for kj in (ki - 1, ki):
                        kk = KERNELS[kj]
                        nc.vector.tensor_reduce(
                            out=ACC[0 : 129 - kk, 1, kj : kj + 1],
                            in_=dxv[0 : 129 - kk, kj - ki + 1, :, NB - 1 :],
                            axis=mybir.AxisListType.XY,
                            op=OP.add,
                            apply_absolute_value=True,
                        )
            nc.sync.dma_start(
                out=acc_d[:, :],
                in_=ACC[:, :, :].rearrange("p a k -> p (a k)"),
            )
            if debug:
                nc.sync.dma_start(
                    out=ex_d[:, :],
                    in_=EX[:, :, :, :].rearrange("p a s c -> p (a s c)"),
                )
                nc.sync.dma_start(
                    out=u_d[:, :],
                    in_=U[:, :, :, :].rearrange("p k a c -> p (k a c)"),
                )
                nc.sync.dma_start(
                    out=dx_d[:, :],
                    in_=dx[:, :, :].rearrange("p k c -> p (k c)"),
                )
                nc.sync.dma_start(
                    out=ps_d[:, :],
                    in_=PSC[:, :, :].rearrange("p k c -> p (k c)"),
                )
                nc.sync.dma_start(
                    out=d_d[:, :],
                    in_=D[:, :, :, :].rearrange("p k a c -> p (k a c)"),
                )

    nc.compile()
    return nc


def make_runner(nc):
    """Once-jitted 8-core runner (run_bass_via_pjrt re-traces per call)."""
    import jax
    from jax.sharding import Mesh, PartitionSpec
    from jax.experimental.shard_map import shard_map
    from concourse import bass2jax
    from concourse import mybir as mb

    bass2jax.install_neuronx_cc_hook()

    part_name = nc.partition_id_tensor.name if nc.partition_id_tensor else None
    in_names, out_names, out_avals, zero_outs = [], [], [], []
    for alloc in nc.m.functions[0].allocations:
        if not isinstance(alloc, mb.MemoryLocationSet):
            continue
        name = alloc.memorylocations[0].name
        if alloc.kind == "ExternalInput":
            if name != part_name:
                in_names.append(name)
        elif alloc.kind == "ExternalOutput":
            shape = tuple(alloc.tensor_shape)
            dtype = mb.dt.np(alloc.dtype)
            out_names.append(name)
            out_avals.append(jax.core.ShapedArray(shape, dtype))
            zero_outs.append(np.zeros(shape, dtype))
    n_params = len(in_names)
    all_names = in_names + out_names
    if part_name is not None:
        all_names = all_names + [part_name]
    donate = tuple(range(n_params, n_params + len(out_names)))

    def _body(*args):
        operands = list(args)
        if part_name is not None:
            operands.append(bass2jax.partition_id_tensor())
        outs = bass2jax._bass_exec_p.bind(
            *operands,
            out_avals=tuple(out_avals),
            in_names=tuple(all_names),
            out_names=tuple(out_names),
            lowering_input_output_aliases=(),
            sim_require_finite=True,
            sim_require_nnan=True,
            nc=nc,
        )
        return tuple(outs)

    devices = jax.devices()[:N_CORES]
    mesh = Mesh(np.asarray(devices), ("core",))
    n_args = n_params + len(out_names)
    sharded = jax.jit(
        shard_map(
            _body,
            mesh=mesh,
            in_specs=(PartitionSpec("core"),) * n_args,
            out_specs=(PartitionSpec("core"),) * len(out_names),
            check_rep=False,
        ),
        donate_argnums=donate,
        keep_unused=True,
    )

    def run(in_maps):
        concat_in = [
            np.concatenate([np.asarray(m[name]) for m in in_maps], axis=0)
            for name in in_names
        ]
        concat_zeros = [
            np.zeros((N_CORES * z.shape[0], *z.shape[1:]), z.dtype)
            for z in zero_outs
        ]
        out_arrs = sharded(*concat_in, *concat_zeros)
        out_arrs = [np.asarray(a) for a in out_arrs]
        return [
            {
                name: out_arrs[i].reshape(N_CORES, *out_avals[i].shape)[c]
                for i, name in enumerate(out_names)
            }
            for c in range(N_CORES)
        ]

    return run


def host_layout(a: np.ndarray) -> np.ndarray:
    """[8, 16, 2048] fp32 -> per-core [128, 257] block-transposed + pad."""
    a = a.reshape(N_CORES, ROWS, NB, 128).transpose(0, 3, 1, 2)
    a = a.reshape(N_CORES, 128, COLS)
    out = np.full((N_CORES, 128, NCOL), PAD, dtype=np.float32)
    out[:, :, :COLS] = a
    return np.ascontiguousarray(out)


def make_in_maps(input: np.ndarray, target: np.ndarray):
    x = host_layout(np.ascontiguousarray(input, dtype=np.float32).reshape(
        N_CORES, ROWS, L))
    t = host_layout(np.ascontiguousarray(target, dtype=np.float32).reshape(
        N_CORES, ROWS, L))
    if "w" not in _CACHE:
        _CACHE["w"] = np.ascontiguousarray(
            make_weights().reshape(128, 2 * NK * 128))
    w = _CACHE["w"]
    return [{"x": x[c], "t": t[c], "w": w} for c in range(N_CORES)]


def kernel(input: np.ndarray, target: np.ndarray) -> np.ndarray:
    if "run" not in _CACHE:
        _CACHE["nc"] = build()
        _CACHE["run"] = make_runner(_CACHE["nc"])

    results = _CACHE["run"](make_in_maps(input, target))
    acc = np.stack([r["acc"] for r in results])      # [cores, 128, 12]
    return finish(acc)


def finish(acc: np.ndarray) -> np.ndarray:
    per_k = acc.sum(axis=(0, 1), dtype=np.float64).reshape(2, NK).sum(0)
    counts = np.array([B * C * (L - k + 1) for k in KERNELS], dtype=np.float64)
    return np.float32((per_k / counts).sum())


# revision 21
# speedup vs baseline: 1.0004x; 1.0004x over previous
"""EntropyProfileLoss Trainium2 kernel — TensorE windowed-sum design.

Math: for a window t of length k, sum(softmax(t)*log_softmax(t))
      = S2/S1 - ln(S1),  S1 = sum(exp(t)), S2 = sum(t*exp(t)).

Layout: pure data parallel over batch B=64 -> 8 cores x 8 batches
(16 rows of L=2048 per core).  Each row is split into 16 blocks of
128; the host transposes to a [128, 257] SBUF image whose partition
axis is the position-within-block j and whose free axis is
c = 16*row + block (col 256 is a -100 pad so exp() gives 0).  In this
layout a window sum starting at (j, c) is a sum down the partition
axis spilling into column c+1, which TensorE computes as two banded
matmuls per window size:

  S_k[q, c] = sum_{p=q}^{q+k-1} E[p, c] + sum_{p=0}^{q+k-129} E[p, c+1]
            = (A_k^T E)[q, c]          + (B_k^T E_next)[q, c]

with A_k[p,q] = 1_{q <= p < q+k}, B_k[p,q] = 1_{p < q+k-128} packed as
12 bf16 [128,128] weights (host inputs).  E = exp(x) and XE = x*E are
bf16; the matmuls accumulate S1 (from E) and S2 (from XE) in fp32
PSUM — one 1024-col region [S1x|S2x|S1t|S2t] per k, cycled through
the 8 PSUM banks (4 k's in flight).

Downstream per k: U = ln(S1) (ACT, PSUM-src, fp32), R = exp(-U) bf16
(ACT, batched over k pairs), D = S2*R (DVE, PSUM-src), dU = Ux-Ut
(GpSimd), dD/dx (DVE bf16 2x), invalid-window memsets (GpSimd: block
15 windows with q >= 129-k run past the row end), and a per-k-pair
|dx| sum via tensor_reduce(apply_absolute_value) into ACC[128, 6].
The host reduces ACC over cores/partitions and applies the per-k
1/(B*C*(L-k+1)) mean scaling.

DMA: x on the sync queue, t + the first 4 weights on the scalar
queue, remaining weights on sync after x — two HWDGE queues in
parallel.  ACT runs Exp and Ln from the single
natural_log_exp_and_others table set (see _patch_act_tables).
"""

import sys

import numpy as np

if "/opt/trn_rl_repo" not in sys.path:
    sys.path.insert(0, "/opt/trn_rl_repo")

import ml_dtypes

import concourse.bacc as bacc
import concourse.bass as bass
import concourse.tile as tile
from concourse import mybir

KERNELS = (4, 8, 16, 32, 64, 128)
NK = len(KERNELS)
B, C, L = 64, 2, 2048
N_CORES = 8
ROWS = (B // N_CORES) * C          # 16 rows per core
NB = L // 128                      # 16 blocks per row
COLS = ROWS * NB                   # 256 (free dim: c = 16*row + block)
NCOL = COLS + 1                    # +1 pad col (= -100 -> exp = 0)
PAD = -100.0

F32 = mybir.dt.float32
BF16 = mybir.dt.bfloat16
AF = mybir.ActivationFunctionType
OP = mybir.AluOpType

_CACHE: dict = {}


def _patch_act_tables():
    """Keep Exp/Ln resolvable only via natural_log_exp_and_others so the
    table-load pass emits one ACT table set (one ~2.7us load)."""
    if _CACHE.get("act_patched"):
        return
    orig = bacc.get_activation_tables
    funcs = {AF.Exp, AF.Ln}

    def patched(arch):
        tables = dict(orig(arch))
        return {
            name: (fs if name == "natural_log_exp_and_others" else fs - funcs)
            for name, fs in tables.items()
        }

    bacc.get_activation_tables = patched
    _CACHE["act_patched"] = True


def make_weights() -> np.ndarray:
    """[128, 2*NK, 128] bf16: A_k, B_k interleaved in k order."""
    p = np.arange(128)[:, None]
    q = np.arange(128)[None, :]
    w = np.zeros((128, 2 * NK, 128), dtype=np.float32)
    for ki, k in enumerate(KERNELS):
        w[:, 2 * ki, :] = ((q <= p) & (p < q + k)).astype(np.float32)
        w[:, 2 * ki + 1, :] = (p < q + k - 128).astype(np.float32)
    return w.astype(ml_dtypes.bfloat16)


def build(debug: bool = False, xe_on_gpsimd: bool = False,
          du_on_gpsimd: bool = True, warm_mms: int = 18):
    _patch_act_tables()
    nc = bacc.Bacc("TRN2", target_bir_lowering=False)

    x_d = nc.dram_tensor("x", [128, NCOL], F32, kind="ExternalInput")
    t_d = nc.dram_tensor("t", [128, NCOL], F32, kind="ExternalInput")
    w_d = nc.dram_tensor("w", [128, 2 * NK * 128], BF16, kind="ExternalInput")
    acc_d = nc.dram_tensor("acc", [128, 2 * NK], F32, kind="ExternalOutput")
    if debug:
        ex_d = nc.dram_tensor("dbg_ex", [128, 4 * NCOL], BF16,
                              kind="ExternalOutput")
        u_d = nc.dram_tensor("dbg_u", [128, NK * 2 * COLS], F32,
                             kind="ExternalOutput")
        dx_d = nc.dram_tensor("dbg_dx", [128, NK * COLS], BF16,
                              kind="ExternalOutput")
        ps_d = nc.dram_tensor("dbg_ps", [128, NK * 1024], F32,
                              kind="ExternalOutput")
        d_d = nc.dram_tensor("dbg_d", [128, NK * 2 * COLS], BF16,
                             kind="ExternalOutput")

    with tile.TileContext(nc) as tc:
        with (
            tc.tile_pool(name="big", bufs=1) as big,
            tc.tile_pool(name="ps", bufs=1, space="PSUM") as psp,
        ):
            X = big.tile([128, 2, NCOL], F32)          # [x | t]
            EX = big.tile([128, 2, 2, NCOL], BF16)     # [tensor][E | XE]
            W = big.tile([128, 2 * NK, 128], BF16)
            U = big.tile([128, NK, 2, COLS], F32)
            R = big.tile([128, NK, 2, COLS], BF16)
            D = big.tile([128, NK, 2, COLS], BF16)
            dU = big.tile([128, NK, COLS], BF16)
            dD = big.tile([128, NK, COLS], BF16)
            dx = big.tile([128, NK, COLS], BF16)
            ACC = big.tile([128, 2, NK], F32)          # [main | block-15 tail]
            PS = psp.tile([128, 4096], F32)            # all 8 banks

            WARM = big.tile([128, 128], BF16)
            DUMO = big.tile([128, 1], F32)
            # dummy activation: anchors the ACT table load ahead of the
            # scalar-queue DMA issues, so exp isn't gated on a late load
            nc.vector.memset(WARM[:, :], 0.0)
            nc.scalar.activation(out=DUMO[:, :], in_=WARM[:, 0:1], func=AF.Exp)
            nc.vector.memset(ACC[:, :, :], 0.0)
            PSC = big.tile([128, NK, 1024], F32, name="PSC") if debug else None

            # ---- input DMA: x/t split across both HWDGE queues, weights
            # on the GpSimd SWDGE queue (k=4,8 first so matmuls start early)
            wv = W[:, :, :].rearrange("p a b -> p (a b)")
            nc.gpsimd.dma_start(out=wv[:, 0 : 4 * 128], in_=w_d[:, 0 : 4 * 128])
            nc.gpsimd.dma_start(out=wv[:, 4 * 128 :], in_=w_d[:, 4 * 128 :])
            nc.sync.dma_start(out=X[0:64, 0, :], in_=x_d[0:64, :])
            nc.scalar.dma_start(out=X[64:128, 0, :], in_=x_d[64:128, :])
            nc.sync.dma_start(out=X[0:64, 1, :], in_=t_d[0:64, :])
            nc.scalar.dma_start(out=X[64:128, 1, :], in_=t_d[64:128, :])

            # HAM warmup: keep the PE busy during the DMA wait so the real
            # matmuls run at 2.4 GHz (unthrottled) — scratch region in the
            # k=32 slot, overwritten by its start=True matmul later
            for _ in range(warm_mms):
                nc.tensor.matmul(
                    PS[:, 3584:3712], WARM[:, :], WARM[:, :],
                    start=True, stop=True,
                )

            # ---- E = exp(x) bf16, XE = x * E bf16 (pad col -> 0) ----
            for a in range(2):
                nc.scalar.activation(
                    out=EX[:, a, 0, :], in_=X[:, a, :], func=AF.Exp
                )
                xe_eng = nc.gpsimd if xe_on_gpsimd else nc.vector
                xe_eng.tensor_tensor(
                    out=EX[:, a, 1, :], in0=X[:, a, :], in1=EX[:, a, 0, :],
                    op=OP.mult,
                )

            # ---- per window size: banded matmuls + entropy pipeline ----
            for ki, k in enumerate(KERNELS):
                base = (ki % 4) * 1024
                reg = PS[:, base : base + 1024].rearrange(
                    "p (a s c) -> p a s c", a=2, s=2
                )  # [tensor][S1|S2][256]
                # one matmul per (weight, tensor) covering [E | XE] — FD 512
                # fills the bank exactly, so each bank sees one start/stop
                for wi, off, start in ((2 * ki, 0, True), (2 * ki + 1, 1, False)):
                    for a in range(2):
                        nc.tensor.matmul(
                            reg[:, a, :, :],
                            W[:, wi, :],
                            EX[:, a, :, off : off + COLS],
                            start=start,
                            stop=not start,
                        )
                if debug:
                    nc.vector.tensor_copy(
                        out=PSC[:, ki], in_=PS[:, base : base + 1024]
                    )
                nc.scalar.activation(
                    out=U[:, ki], in_=reg[:, :, 0, :], func=AF.Ln
                )
                du_eng = nc.gpsimd if du_on_gpsimd else nc.vector
                du_eng.tensor_tensor(
                    out=dU[:, ki], in0=U[:, ki, 0], in1=U[:, ki, 1],
                    op=OP.subtract,
                )
                # R = exp(-U), then D = S2*R straight from PSUM (frees slot)
                nc.scalar.activation(
                    out=R[:, ki], in_=U[:, ki], func=AF.Exp, scale=-1.0,
                )
                nc.vector.tensor_tensor(
                    out=D[:, ki], in0=reg[:, :, 1, :], in1=R[:, ki],
                    op=OP.mult,
                )
                if ki % 2 == 1:
                    nc.vector.tensor_tensor(
                        out=dD[:, ki - 1 : ki + 1],
                        in0=D[:, ki - 1 : ki + 1, 0],
                        in1=D[:, ki - 1 : ki + 1, 1],
                        op=OP.subtract,
                    )
                    nc.vector.tensor_tensor(
                        out=dx[:, ki - 1 : ki + 1],
                        in0=dD[:, ki - 1 : ki + 1],
                        in1=dU[:, ki - 1 : ki + 1],
                        op=OP.subtract,
                    )
                    # |dx| sums, skipping invalid windows (block 15 windows
                    # with q >= 129-k run past the row end): main reduce over
                    # blocks 0-14, tail reduce over block 15's valid rows
                    dxv = dx[:, ki - 1 : ki + 1].rearrange(
                        "p kk (r b) -> p kk r b", b=NB
                    )
                    nc.vector.tensor_reduce(
                        out=ACC[:, 0, ki - 1 : ki + 1],
                        in_=dxv[:, :, :, 0 : NB - 1],
                        axis=mybir.AxisListType.XY,
                        op=OP.add,
                        apply_absolute_value=True,
                    )
                    for kj in (ki - 1, ki):
                        kk = KERNELS[kj]
                        nc.vector.tensor_reduce(
                            out=ACC[0 : 129 - kk, 1, kj : kj + 1],
                            in_=dxv[0 : 129 - kk, kj - ki + 1, :, NB - 1 :],
                            axis=mybir.AxisListType.XY,
                            op=OP.add,
                            apply_absolute_value=True,
                        )
            nc.sync.dma_start(
                out=acc_d[:, :],
                in_=ACC[:, :, :].rearrange("p a k -> p (a k)"),
            )
            if debug:
                nc.sync.dma_start(
                    out=ex_d[:, :],
                    in_=EX[:, :, :, :].rearrange("p a s c -> p (a s c)"),
                )
                nc.sync.dma_start(
                    out=u_d[:, :],
                    in_=U[:, :, :, :].rearrange("p k a c -> p (k a c)"),
                )
                nc.sync.dma_start(
                    out=dx_d[:, :],
                    in_=dx[:, :, :].rearrange("p k c -> p (k c)"),
                )
                nc.sync.dma_start(
                    out=ps_d[:, :],
                    in_=PSC[:, :, :].rearrange("p k c -> p (k c)"),
                )
                nc.sync.dma_start(
                    out=d_d[:, :],
                    in_=D[:, :, :, :].rearrange("p k a c -> p (k a c)"),
                )

    nc.compile()
    return nc


def make_runner(nc):
    """Once-jitted 8-core runner (run_bass_via_pjrt re-traces per call)."""
    import jax
    from jax.sharding import Mesh, PartitionSpec
    from jax.experimental.shard_map import shard_map
    from concourse import bass2jax
    from concourse import mybir as mb

    bass2jax.install_neuronx_cc_hook()

    part_name = nc.partition_id_tensor.name if nc.partition_id_tensor else None
    in_names, out_names, out_avals, zero_outs = [], [], [], []
    for alloc in nc.m.functions[0].allocations:
        if not isinstance(alloc, mb.MemoryLocationSet):
            continue
        name = alloc.memorylocations[0].name
        if alloc.kind == "ExternalInput":
            if name != part_name:
                in_names.append(name)
        elif alloc.kind == "ExternalOutput":
            shape = tuple(alloc.tensor_shape)
            dtype = mb.dt.np(alloc.dtype)
            out_names.append(name)
            out_avals.append(jax.core.ShapedArray(shape, dtype))
            zero_outs.append(np.zeros(shape, dtype))
    n_params = len(in_names)
    all_names = in_names + out_names
    if part_name is not None:
        all_names = all_names + [part_name]
    donate = tuple(range(n_params, n_params + len(out_names)))

    def _body(*args):
        operands = list(args)
        if part_name is not None:
            operands.append(bass2jax.partition_id_tensor())
        outs = bass2jax._bass_exec_p.bind(
            *operands,
            out_avals=tuple(out_avals),
            in_names=tuple(all_names),
            out_names=tuple(out_names),
            lowering_input_output_aliases=(),
            sim_require_finite=True,
            sim_require_nnan=True,
            nc=nc,
        )
        return tuple(outs)

    devices = jax.devices()[:N_CORES]
    mesh = Mesh(np.asarray(devices), ("core",))
    n_args = n_params + len(out_names)
    sharded = jax.jit(
        shard_map(
            _body,
            mesh=mesh,
            in_specs=(PartitionSpec("core"),) * n_args,
            out_specs=(PartitionSpec("core"),) * len(out_names),
            check_rep=False,
        ),
        donate_argnums=donate,
        keep_unused=True,
    )

    def run(in_maps):
        concat_in = [
            np.concatenate([np.asarray(m[name]) for m in in_maps], axis=0)
            for name in in_names
        ]
        concat_zeros = [
            np.zeros((N_CORES * z.shape[0], *z.shape[1:]), z.dtype)
            for z in zero_outs
        ]
        out_arrs = sharded(*concat_in, *concat_zeros)
        out_arrs = [np.asarray(a) for a in out_arrs]
        return [
            {
                name: out_arrs[i].reshape(N_CORES, *out_avals[i].shape)[c]
                for i, name in enumerate(out_names)
            }
            for c in range(N_CORES)
        ]

    return run


def host_layout(a: np.ndarray) -> np.ndarray:
    """[8, 16, 2048] fp32 -> per-core [128, 257] block-transposed + pad."""
    a = a.reshape(N_CORES, ROWS, NB, 128).transpose(0, 3, 1, 2)
    a = a.reshape(N_CORES, 128, COLS)
    out = np.full((N_CORES, 128, NCOL), PAD, dtype=np.float32)
    out[:, :, :COLS] = a
    return np.ascontiguousarray(out)


def make_in_maps(input: np.ndarray, target: np.ndarray):
    x = host_layout(np.ascontiguousarray(input, dtype=np.float32).reshape(
        N_CORES, ROWS, L))
    t = host_layout(np.ascontiguousarray(target, dtype=np.float32).reshape(
        N_CORES, ROWS, L))
    if "w" not in _CACHE:
        _CACHE["w"] = np.ascontiguousarray(
            make_weights().reshape(128, 2 * NK * 128))
    w = _CACHE["w"]
    return [{"x": x[c], "t": t[c], "w": w} for c in range(N_CORES)]


def kernel(input: np.ndarray, target: np.ndarray) -> np.ndarray:
    if "run" not in _CACHE:
        _CACHE["nc"] = build()
        _CACHE["run"] = make_runner(_CACHE["nc"])

    results = _CACHE["run"](make_in_maps(input, target))
    acc = np.stack([r["acc"] for r in results])      # [cores, 128, 12]
    return finish(acc)


def finish(acc: np.ndarray) -> np.ndarray:
    per_k = acc.sum(axis=(0, 1), dtype=np.float64).reshape(2, NK).sum(0)
    counts = np.array([B * C * (L - k + 1) for k in KERNELS], dtype=np.float64)
    return np.float32((per_k / counts).sum())
